# revision 1
# baseline (speedup 1.0000x reference)
"""Bidirectional DragnnLSTM kernel for 8 Trainium2 NeuronCores.

Sharding: cores 0-3 run the forward (lr) direction, cores 4-7 the backward
(rl) direction (they receive x reversed).  Within each quad, the hidden
dimension H=1536 of every gate matrix is column-sharded 4 ways (384 cols per
core).  Each recurrence step computes the local 384-wide slice of every gate
via f32r GEMV chains on the TensorEngine, then the cell/hidden slices are
exchanged between quad peers with remote SBUF-to-SBUF DMA broadcasts
(~1-2us) instead of collective_compute (~5-10us floor).

h-space permutation: element e of h lives at SBUF [p, k] with
p = (e % 384) // 3, k = 3*(e // 384) + (e % 3).  This makes the
[1,384] gate-row -> [128,3] column-block reshape a contiguous DMA on both
sides; all h/c-contracting weight matrices have their rows permuted
identically on the host, which costs nothing.

The per-step x-projections (x @ x2g + b) are precomputed for all steps with
regular matmuls (bias folded in via an appended ones-row), stored as
[128,3,384] tiles, and added into each step's PSUM accumulation with an
identity-column row-select matmul.

After the recurrence every core has written its h-slice history [384,384]
(h-dims x tokens) to HBM; one AllGather over all 8 cores produces
ff_in.T = [3072, 384] everywhere, and each core redundantly computes the
feed-forward + head (hidden kept transposed so biases are per-partition),
writing the full logits [384, 128].
"""

import sys
import numpy as np

sys.path.insert(0, "/opt/trn_rl_repo")

SEQ = 384
D_IN = 768
H = 1536
FF = 768
NA = 128
G = 4            # cores per direction
CPG = H // G     # 384 columns per core
KH = H // 128    # 12 contraction chunks
KX = 7           # x-side chunks: ceil(769/128)
KF = 24          # ff contraction chunks (3072/128)
MF = FF // 128   # 6 ff output chunks
TC = SEQ // 128  # 3 token chunks

_CACHE = {}


def _build(seq, skip_hist=False):
    import concourse.bass as bass
    import concourse.bacc as bacc
    import concourse.mybir as mybir

    F32 = mybir.dt.float32
    F32R = mybir.dt.float32r
    U32 = mybir.dt.uint32
    AF = mybir.ActivationFunctionType
    tc_n = seq // 128 if seq >= 128 else 1  # token chunks (seq assumed mult of 128 or < 128)
    tchunks = [(i * 128, min(128, seq - i * 128)) for i in range(((seq + 127) // 128))]

    nc = bacc.Bacc(target_bir_lowering=False)

    # ---------------- DRAM I/O ----------------
    xT_d = nc.dram_tensor("xT", [128, KX, seq], F32R, kind="ExternalInput")
    xw_d = {g: nc.dram_tensor(f"xw_{g}", [128, KX, CPG], F32R, kind="ExternalInput")
            for g in "ico"}
    W_d = {m: nc.dram_tensor(f"W_{m}", [128, KH, CPG], F32R, kind="ExternalInput")
           for m in ("hi", "ci", "hc", "ho", "co")}
    ident_d = nc.dram_tensor("ident", [128, 128], F32R, kind="ExternalInput")
    onesrow_d = nc.dram_tensor("onesrow", [128, 128], F32R, kind="ExternalInput")
    hb1_d = nc.dram_tensor("hb1", [128, NA], F32R, kind="ExternalInput")
    ffw_d = nc.dram_tensor("ffw", [128, KF, FF], F32R, kind="ExternalInput")
    ffb_d = nc.dram_tensor("ffb", [128, MF], F32, kind="ExternalInput")
    hw_d = nc.dram_tensor("hw", [128, MF, NA], F32R, kind="ExternalInput")

    out_d = nc.dram_tensor("logits", [seq, NA], F32, kind="ExternalOutput")

    h_hist_f = nc.dram_tensor("h_hist_f", [CPG, seq], F32R)   # my h-dims x tokens
    h_hist_r = nc.dram_tensor("h_hist_r", [CPG, seq], F32R)   # token-reversed copy
    ag_out_f = nc.dram_tensor("ag_out_f", [2 * H, seq], F32R, addr_space="Shared")
    ag_out_r = nc.dram_tensor("ag_out_r", [2 * H, seq], F32R, addr_space="Shared")

    # ---------------- SBUF ----------------
    overlay_base = nc.sbuf_base  # FF tensors will overlay the recurrence weights
    xT_s = nc.alloc_sbuf_tensor("xT_s", [128, KX, seq], F32R)
    xw_s = {g: nc.alloc_sbuf_tensor(f"xw_{g}_s", [128, KX, CPG], F32R) for g in "ico"}
    W_s = {m: nc.alloc_sbuf_tensor(f"W_{m}_s", [128, KH, CPG], F32R)
           for m in ("hi", "ci", "hc", "ho", "co")}
    ident_s = nc.alloc_sbuf_tensor("ident_s", [128, 128], F32R)
    onesrow_s = nc.alloc_sbuf_tensor("onesrow_s", [128, 128], F32R)
    hb1_s = nc.alloc_sbuf_tensor("hb1_s", [128, NA], F32R)
    ffb_s = nc.alloc_sbuf_tensor("ffb_s", [128, MF], F32)

    xp_s = {g: nc.alloc_sbuf_tensor(f"xp_{g}_s", [128, len(tchunks), CPG], F32R)
            for g in "ico"}

    h_full = [nc.alloc_sbuf_tensor(f"h_full{p}", [128, KH], F32R) for p in range(2)]
    c_full = [nc.alloc_sbuf_tensor(f"c_full{p}", [128, KH], F32R) for p in range(2)]

    row = lambda name: nc.alloc_sbuf_tensor(name, [1, CPG], F32)
    it_r, wt_r, tmp_r, tmp2_r, tct_r, ot_r = (
        row("it_r"), row("wt_r"), row("tmp_r"), row("tmp2_r"), row("tct_r"), row("ot_r"))
    ct_r = [nc.alloc_sbuf_tensor(f"ct_r{p}", [1, CPG], F32R) for p in range(2)]
    ht_r = [nc.alloc_sbuf_tensor(f"ht_r{p}", [1, CPG], F32R) for p in range(2)]

    # FF stage tensors overlay the recurrence weight region (phases are
    # strictly sequential; loads gated on VHT >= seq)
    off0 = (overlay_base + 31) & ~31
    ffw_s = nc.alloc_sbuf_tensor_at("ffw_s", [128, KF, FF], F32R, offset=off0)
    off0 += KF * FF * 4
    ag_s = nc.alloc_sbuf_tensor_at("ag_s", [128, KF, seq], F32R, offset=off0)
    off0 += KF * seq * 4
    hidT_s = nc.alloc_sbuf_tensor_at("hidT_s", [128, MF, seq], F32R, offset=off0)
    hw_s = nc.alloc_sbuf_tensor("hw_s", [128, MF, NA], F32R)
    log_s = nc.alloc_sbuf_tensor("log_s", [128, len(tchunks), NA], F32)

    pb = [nc.place_psum_tensor(f"pb{i}", [128, 448], F32, bank=i) for i in range(8)]

    NLOADS = 1 + 3 + 5 + 3 + 4 + 2  # xT, xw*3, W*5, ident/onesrow/hb1, ffw/ffb/hw/..., myoff/isrl
    # count precisely below instead
    sems = {}
    semnames = ["LD", "XPMM", "XPCP", "MM1", "MM2", "VE1", "VCT", "VHT",
                "AC1", "ATCT", "AC2", "LCT", "LHT", "HIST", "PSEM", "BLS",
                "RCT", "RHT", "CC", "FFMM", "RELU", "HCPY", "LD2", "OUTD"]

    import contextlib
    ctx = contextlib.ExitStack()
    for s in semnames:
        sems[s] = ctx.enter_context(nc.semaphore(s))
    S = type("S", (), sems)

    load_list = [
        (xT_s.ap(), xT_d.ap()),
        (xw_s["i"].ap(), xw_d["i"].ap()), (xw_s["c"].ap(), xw_d["c"].ap()),
        (xw_s["o"].ap(), xw_d["o"].ap()),
        (W_s["hi"].ap(), W_d["hi"].ap()), (W_s["ci"].ap(), W_d["ci"].ap()),
        (W_s["hc"].ap(), W_d["hc"].ap()), (W_s["ho"].ap(), W_d["ho"].ap()),
        (W_s["co"].ap(), W_d["co"].ap()),
        (ident_s[:, :], ident_d[:, :]), (onesrow_s[:, :], onesrow_d[:, :]),
        (hb1_s[:, :], hb1_d[:, :]), (ffb_s[:, :], ffb_d[:, :]),
        (hw_s.ap(), hw_d.ap()),
    ]
    NLOADS = len(load_list)

    # xproj chains: list of (gate, tchunk_idx)
    xp_chains = [(g, i) for i in range(len(tchunks)) for g in "ico"]
    NXP = len(xp_chains)

    block_ctx = nc.Block()
    block = block_ctx.__enter__()

    # =================== SYNC: loads + h_hist + output ===================
    @block.sync
    def _(sync):
        for dst, src in load_list:
            sync.dma_start(out=dst, in_=src).then_inc(S.LD, 16)

        with nc.allow_non_contiguous_dma(reason="h_hist column scatter"):
            for t in range(seq):
                sync.wait_ge(S.VHT, t + 1)
                # both orders written; the right half is picked statically after AG
                src = ht_r[t % 2][0:1, :]
                if skip_hist:
                    sync.dma_start(out=bass.AP(h_hist_f, 0, [[seq, 1], [1, CPG]]),
                                   in_=src).then_inc(S.HIST, 16)
                    sync.dma_start(out=bass.AP(h_hist_r, 0, [[seq, 1], [1, CPG]]),
                                   in_=src).then_inc(S.HIST, 16)
                else:
                    sync.dma_start(out=bass.AP(h_hist_f, t, [[seq, CPG], [1, 1]]),
                                   in_=src).then_inc(S.HIST, 16)
                    sync.dma_start(out=bass.AP(h_hist_r, seq - 1 - t, [[seq, CPG], [1, 1]]),
                                   in_=src).then_inc(S.HIST, 16)

        # FF loads: ffw overlays the weight region -> wait until the
        # recurrence has fully consumed the weights
        sync.wait_ge(S.VHT, seq)
        sync.wait_ge(S.MM2, seq)
        sync.dma_start(out=ffw_s.ap(), in_=ffw_d.ap()).then_inc(S.LD2, 16)
        sync.wait_ge(S.CC, 2)
        half = KF // 2
        sync.dma_start(
            out=ag_s[:, 0:half, :],
            in_=bass.AP(ag_out_f, 0, [[seq, 128], [128 * seq, half], [1, seq]]),
        ).then_inc(S.LD2, 16)
        sync.dma_start(
            out=ag_s[:, half:KF, :],
            in_=bass.AP(ag_out_r, H * seq, [[seq, 128], [128 * seq, half], [1, seq]]),
        ).then_inc(S.LD2, 16)
        # LD2 total: ffw (16) + ag halves (32) = 48
        # final output
        for i, (t0, tl) in enumerate(tchunks):
            sync.wait_ge(S.HCPY, i + 1)
            sync.dma_start(
                out=bass.AP(out_d, t0 * NA, [[NA, tl], [1, NA]]),
                in_=log_s[0:tl, i, :],
            ).then_inc(S.OUTD, 16)
        sync.wait_ge(S.OUTD, 16 * len(tchunks))

    # =================== TENSOR ===================
    @block.tensor
    def _(tensor):
        tensor.wait_ge(S.LD, 16 * NLOADS)

        # ---- x projections ----
        for idx, (g, i) in enumerate(xp_chains):
            b = idx % 8
            if idx >= 8:
                tensor.wait_ge(S.XPCP, idx - 7)
            t0, tl = tchunks[i]
            for k in range(KX):
                mm = tensor.matmul(pb[b][0:tl, 0:CPG],
                                   xT_s[:, k, t0:t0 + tl],
                                   xw_s[g][:, k, :],
                                   start=(k == 0), stop=(k == KX - 1),
                                   skip_group_check=True)
            mm.then_inc(S.XPMM, 1)

        tensor.wait_ge(S.XPCP, NXP)

        # ---- recurrence ----
        for t in range(seq):
            par = t % 2
            prev = (t - 1) % 2
            B = [pb[4 * par + i] for i in range(4)]  # b0..b3 this parity
            if t >= 2:
                tensor.wait_ge(S.VHT, t - 1)

            tci = t // 128
            trow = t % 128
            tlc = tchunks[tci][1]

            # b0: c2i (runs in prior step's shadow) + h2i + xp_i select,
            # all one serial accumulation chain on col-group 0
            if t >= 1:
                tensor.wait_ge(S.RCT, 6 * t)
                tensor.wait_ge(S.LCT, 16 * t)
                for k in range(KH):
                    tensor.matmul(B[0][0:1, 0:CPG], c_full[prev][:, k:k + 1],
                                  W_s["ci"][:, k, :],
                                  start=(k == 0), stop=False,
                                  tile_position=(0, 0), skip_group_check=True)
                tensor.wait_ge(S.RHT, 6 * t)
                tensor.wait_ge(S.LHT, 16 * t)
                for k in range(KH):
                    tensor.matmul(B[0][0:1, 0:CPG], h_full[prev][:, k:k + 1],
                                  W_s["hi"][:, k, :],
                                  start=False, stop=False,
                                  tile_position=(0, 0), skip_group_check=True)
            tensor.matmul(B[0][0:1, 0:CPG], ident_s[0:tlc, trow:trow + 1],
                          xp_s["i"][0:tlc, tci, :],
                          start=(t == 0), stop=True,
                          tile_position=(0, 0), skip_group_check=True)

            # b2: h2c + xp_c select
            if t >= 1:
                for k in range(KH):
                    tensor.matmul(B[2][0:1, 0:CPG], h_full[prev][:, k:k + 1],
                                  W_s["hc"][:, k, :],
                                  start=(k == 0), stop=False,
                                  tile_position=(0, 0), skip_group_check=True)
            tensor.matmul(B[2][0:1, 0:CPG], ident_s[0:tlc, trow:trow + 1],
                          xp_s["c"][0:tlc, tci, :],
                          start=(t == 0), stop=True,
                          tile_position=(0, 0), skip_group_check=True)

            # b3 part 1: h2o + xp_o select (accumulation group stays open)
            if t >= 1:
                for k in range(KH):
                    tensor.matmul(B[3][0:1, 0:CPG], h_full[prev][:, k:k + 1],
                                  W_s["ho"][:, k, :],
                                  start=(k == 0), stop=False,
                                  tile_position=(0, 0), skip_group_check=True)
            tensor.matmul(B[3][0:1, 0:CPG], ident_s[0:tlc, trow:trow + 1],
                          xp_s["o"][0:tlc, tci, :],
                          start=(t == 0), stop=False,
                          tile_position=(0, 0), skip_group_check=True
                          ).then_inc(S.MM1, 1)

            # b3 part 2: c2o on fresh ct
            tensor.wait_ge(S.RCT, 6 * (t + 1))
            tensor.wait_ge(S.LCT, 16 * (t + 1))
            for k in range(KH):
                mm = tensor.matmul(B[3][0:1, 0:CPG], c_full[par][:, k:k + 1],
                                   W_s["co"][:, k, :],
                                   start=False, stop=(k == KH - 1),
                                   tile_position=(0, 0), skip_group_check=True)
            mm.then_inc(S.MM2, 1)

        # ---- FF ----
        tensor.wait_ge(S.VHT, seq)
        tensor.wait_ge(S.LD2, 48)
        for m in range(MF):
            for k in range(KF):
                mm = tensor.matmul(pb[m][0:128, 0:seq],
                                   ffw_s[:, k, 128 * m:128 * (m + 1)],
                                   ag_s[:, k, :],
                                   start=(k == 0), stop=(k == KF - 1),
                                   skip_group_check=True)
            mm.then_inc(S.FFMM, 1)
        tensor.wait_ge(S.RELU, MF)
        for i, (t0, tl) in enumerate(tchunks):
            b = pb[6 + (i % 2)]
            if i >= 2:
                tensor.wait_ge(S.HCPY, i - 1)
            for k in range(MF):
                tensor.matmul(b[0:tl, 0:NA], hidT_s[:, k, t0:t0 + tl],
                              hw_s[:, k, :],
                              start=(k == 0), stop=False, skip_group_check=True)
            tensor.matmul(b[0:tl, 0:NA], onesrow_s[:, 0:tl], hb1_s[:, :],
                          start=False, stop=True, skip_group_check=True
                          ).then_inc(S.MM1, 1)  # reuse MM1 as head-chain-done

    # =================== VECTOR ===================
    @block.vector
    def _(vector):
        # xproj copies
        for idx, (g, i) in enumerate(xp_chains):
            b = idx % 8
            vector.wait_ge(S.XPMM, idx + 1)
            t0, tl = tchunks[i]
            vector.tensor_copy(xp_s[g][0:tl, i, :],
                               pb[b][0:tl, 0:CPG]).then_inc(S.XPCP, 1)

        for t in range(seq):
            par = t % 2
            prev = (t - 1) % 2
            B = [pb[4 * par + i] for i in range(4)]
            vector.wait_ge(S.AC1, t + 1)
            if t >= 2:
                vector.wait_ge(S.LCT, 16 * t)
            if t == 0:
                vector.tensor_mul(ct_r[0][0:1, :], it_r[0:1, :], wt_r[0:1, :]
                                  ).then_inc(S.VCT, 1)
            else:
                vector.tensor_sub(tmp_r[0:1, :], wt_r[0:1, :],
                                  ct_r[prev][0:1, :].bitcast(mybir.dt.float32))
                vector.drain()
                vector.tensor_mul(tmp2_r[0:1, :], it_r[0:1, :], tmp_r[0:1, :])
                vector.drain()
                vector.tensor_add(ct_r[par][0:1, :], tmp2_r[0:1, :],
                                  ct_r[prev][0:1, :].bitcast(mybir.dt.float32)
                                  ).then_inc(S.VCT, 1)
            vector.wait_ge(S.ATCT, t + 1)
            vector.wait_ge(S.AC2, t + 1)
            if t >= 2:
                vector.wait_ge(S.LHT, 16 * t)
                vector.wait_ge(S.HIST, 32 * t)
            vector.tensor_mul(ht_r[par][0:1, :], ot_r[0:1, :], tct_r[0:1, :]
                              ).then_inc(S.VHT, 1)

        # head copies
        for i, (t0, tl) in enumerate(tchunks):
            b = pb[6 + (i % 2)]
            vector.wait_ge(S.MM1, seq + i + 1)
            vector.tensor_copy(log_s[0:tl, i, :], b[0:tl, 0:NA]).then_inc(S.HCPY, 1)

    # =================== SCALAR (ACT) ===================
    @block.scalar
    def _(scalar):
        for t in range(seq):
            par = t % 2
            B = [pb[4 * par + i] for i in range(4)]
            scalar.wait_ge(S.MM1, t + 1)
            if t >= 1:
                scalar.wait_ge(S.VCT, t)
            scalar.activation(wt_r[0:1, :], B[2][0:1, 0:CPG], AF.Tanh)
            scalar.activation(it_r[0:1, :], B[0][0:1, 0:CPG], AF.Sigmoid
                              ).then_inc(S.AC1, 1)
            scalar.wait_ge(S.VCT, t + 1)
            scalar.activation(tct_r[0:1, :], ct_r[par][0:1, :].bitcast(mybir.dt.float32),
                              AF.Tanh).then_inc(S.ATCT, 1)
            scalar.wait_ge(S.MM2, t + 1)
            scalar.activation(ot_r[0:1, :], B[3][0:1, 0:CPG], AF.Sigmoid
                              ).then_inc(S.AC2, 1)

        # FF relu with per-partition bias
        for m in range(MF):
            scalar.wait_ge(S.FFMM, m + 1)
            scalar.activation(hidT_s[:, m, :],
                              pb[m][0:128, 0:seq], AF.Relu,
                              bias=ffb_s[:, m:m + 1]).then_inc(S.RELU, 1)

    # =================== GPSIMD: scatter + remote exchange + collective ===================
    @block.gpsimd
    def _(gpsimd):
        gpsimd.wait_ge(S.LD, 16 * NLOADS)
        for t in range(seq):
            par = t % 2
            # ct scatter (my slab lives at cols 0:3 on every core) + 3 XOR-slot bcasts
            gpsimd.wait_ge(S.VCT, t + 1)
            if t >= 1:
                gpsimd.wait_ge(S.BLS, 16 * 6 * t)           # all bcasts thru ht(t-1)
            gpsimd.dma_start(
                out=c_full[par][:, 0:3],
                in_=bass.AP(ct_r[par], 0, [[CPG, 1], [3, 128], [1, 3]]),
            ).then_inc(S.LCT, 16)
            gpsimd.wait_ge(S.LCT, 16 * (t + 1))
            for delta in (1, 2, 3):
                rd = [None] * 8
                rd[delta] = (0, delta)
                gpsimd.remote_dma_broadcast(
                    out_ap=c_full[par][:, 3 * delta:3 * delta + 3],
                    in_ap=c_full[par][:, 0:3],
                    remote_sem=S.RCT, local_sem=S.BLS,
                    rdests=rd,
                ).then_inc(S.PSEM, 1)
            gpsimd.wait_ge(S.PSEM, 6 * t + 3)
            gpsimd.trigger_dma(3)

            # ht scatter + bcasts
            gpsimd.wait_ge(S.VHT, t + 1)
            gpsimd.wait_ge(S.BLS, 16 * (6 * t + 3))         # all bcasts thru ct(t)
            gpsimd.dma_start(
                out=h_full[par][:, 0:3],
                in_=bass.AP(ht_r[par], 0, [[CPG, 1], [3, 128], [1, 3]]),
            ).then_inc(S.LHT, 16)
            gpsimd.wait_ge(S.LHT, 16 * (t + 1))
            for delta in (1, 2, 3):
                rd = [None] * 8
                rd[delta] = (0, delta)
                gpsimd.remote_dma_broadcast(
                    out_ap=h_full[par][:, 3 * delta:3 * delta + 3],
                    in_ap=h_full[par][:, 0:3],
                    remote_sem=S.RHT, local_sem=S.BLS,
                    rdests=rd,
                ).then_inc(S.PSEM, 1)
            gpsimd.wait_ge(S.PSEM, 6 * t + 6)
            gpsimd.trigger_dma(3)

        gpsimd.wait_ge(S.HIST, 32 * seq)
        gpsimd.collective_compute(
            "AllGather",
            mybir.AluOpType.bypass,
            replica_groups=[list(range(8))],
            ins=[h_hist_f.ap().opt()],
            outs=[ag_out_f.ap().opt()],
        ).then_inc(S.CC)
        gpsimd.collective_compute(
            "AllGather",
            mybir.AluOpType.bypass,
            replica_groups=[list(range(8))],
            ins=[h_hist_r.ap().opt()],
            outs=[ag_out_r.ap().opt()],
        ).then_inc(S.CC)

    block_ctx.__exit__(None, None, None)
    ctx.close()
    nc.compile()
    return nc


# ---------------- host-side helpers ----------------

def _perm_index(g):
    # E[p, k] = h-space index stored at SBUF [partition p, chunk k] on core
    # with group index g: slot s = k//3 holds the slab of group g ^ s.
    p = np.arange(128)[:, None]
    k = np.arange(KH)[None, :]
    return 384 * ((g ^ (k // 3)) & 3) + 3 * p + (k % 3)


def _prep_core_inputs(c, inp):
    pre = "lr" if c < G else "rl"
    g = c % G
    S = slice(CPG * g, CPG * (g + 1))
    E = _perm_index(g)

    x = np.asarray(inp["x"], np.float32)
    xin = x if pre == "lr" else x[::-1]
    seq = x.shape[0]

    x_aug = np.zeros((KX * 128, seq), np.float32)
    x_aug[:D_IN] = xin.T
    x_aug[D_IN] = 1.0
    xT = np.ascontiguousarray(x_aug.reshape(KX, 128, seq).transpose(1, 0, 2))

    def aug_w(w, b):
        wa = np.zeros((KX * 128, CPG), np.float32)
        wa[:D_IN] = w[:, S]
        wa[D_IN] = b[S]
        return np.ascontiguousarray(wa.reshape(KX, 128, CPG).transpose(1, 0, 2))

    def perm_w(w):
        return np.ascontiguousarray(np.asarray(w, np.float32)[:, S][E])

    d = {
        "xT": xT,
        "xw_i": aug_w(inp[pre + "_x2i"], inp[pre + "_bi"]),
        "xw_c": aug_w(inp[pre + "_x2c"], inp[pre + "_bc"]),
        "xw_o": aug_w(inp[pre + "_x2o"], inp[pre + "_bo"]),
        "W_hi": perm_w(inp[pre + "_h2i"]),
        "W_ci": perm_w(inp[pre + "_c2i"]),
        "W_hc": perm_w(inp[pre + "_h2c"]),
        "W_ho": perm_w(inp[pre + "_h2o"]),
        "W_co": perm_w(inp[pre + "_c2o"]),
        "ident": np.eye(128, dtype=np.float32),
        "onesrow": np.zeros((128, 128), np.float32),
        "hb1": np.zeros((128, NA), np.float32),
        "ffw": np.ascontiguousarray(
            np.asarray(inp["ff_w"], np.float32).reshape(KF, 128, FF).transpose(1, 0, 2)),
        "ffb": np.ascontiguousarray(
            np.asarray(inp["ff_b"], np.float32).reshape(MF, 128).T),
        "hw": np.ascontiguousarray(
            np.asarray(inp["head_w"], np.float32).reshape(MF, 128, NA).transpose(1, 0, 2)),
    }
    d["onesrow"][0, :] = 1.0
    d["hb1"][0, :] = np.asarray(inp["head_b"], np.float32)
    return d


def kernel(**inputs):
    from concourse.bass_utils import run_bass_kernel_spmd

    seq = np.asarray(inputs["x"]).shape[0]
    if "nc" not in _CACHE or _CACHE.get("seq") != seq:
        _CACHE["nc"] = _build(seq)
        _CACHE["seq"] = seq
    nc = _CACHE["nc"]

    in_maps = [_prep_core_inputs(c, inputs) for c in range(8)]
    res = run_bass_kernel_spmd(nc, in_maps, core_ids=list(range(8)), trace=False)
    return res.results[0]["logits"]



# revision 15
# speedup vs baseline: 4.8216x; 4.8216x over previous
"""Bidirectional DragnnLSTM kernel for 8 Trainium2 NeuronCores.

Sharding: cores 0-3 run the forward (lr) direction, cores 4-7 the backward
(rl) direction (they receive x reversed).  Within each quad, the hidden
dimension H=1536 of every gate matrix is column-sharded 4 ways (384 cols per
core).  Each recurrence step computes the local 384-wide slice of every gate
via f32r GEMV chains on the TensorEngine, then the cell/hidden slices are
exchanged between quad peers with remote SBUF-to-SBUF DMA broadcasts.

h-space permutation: element e of h lives at SBUF [p, k] with
p = (e % 384) // 3, k = 3*(e // 384) + (e % 3).  This makes the
[1,384] gate-row -> [128,3] column-block reshape a contiguous DMA on both
sides; all h/c-contracting weight matrices have their rows permuted
identically on the host, which costs nothing.

The per-step x-projections (x @ x2g + b) are precomputed for all steps with
regular matmuls (bias folded in via an appended ones-row), stored as
[128,3,384] tiles, and added into each step's PSUM accumulation with an
identity-column row-select matmul.

Broadcast descriptors are generated ahead of time on the Pool engine; only
a cheap trigger_dma sits on the critical path.  Scatters run on the sync
(SP) queue.  it/wt activations fire as soon as their own PSUM chains close
(MMA/MMB), letting the h2o chain overlap the ct exchange.

After the recurrence every core has written its h-slice history [384,384]
(h-dims x tokens) to HBM; one AllGather over all 8 cores produces
ff_in.T = [3072, 384] everywhere, and each core redundantly computes the
feed-forward + head (hidden kept transposed so biases are per-partition),
writing the full logits [384, 128].

stage (debug ladder): 0 = PE stream only; 1 = +ACT coupled; 2 = +DVE;
3 = +scatter DMAs; 4 = full (broadcast exchange).  Stages < 4 produce
garbage numerics but valid timing.
"""

import sys
import numpy as np

sys.path.insert(0, "/opt/trn_rl_repo")

SEQ = 384
D_IN = 768
H = 1536
FF = 768
NA = 128
G = 4            # cores per direction
CPG = H // G     # 384 columns per core
KH = H // 128    # 12 contraction chunks
KX = 7           # x-side chunks: ceil(769/128)
KF = 24          # ff contraction chunks (3072/128)
MF = FF // 128   # 6 ff output chunks

_CACHE = {}


def _build(seq, skip_hist=False, sim_solo=False, self_loop=False, local_cc=False,
           stage=4):
    import concourse.bass as bass
    import concourse.bacc as bacc
    import concourse.mybir as mybir

    F32 = mybir.dt.float32
    F32R = mybir.dt.float32r
    AF = mybir.ActivationFunctionType
    tchunks = [(i * 128, min(128, seq - i * 128)) for i in range(((seq + 127) // 128))]

    nc = bacc.Bacc(target_bir_lowering=False)

    # ---------------- DRAM I/O ----------------
    xT_d = nc.dram_tensor("xT", [128, KX, seq], F32R, kind="ExternalInput")
    xw_d = {g: nc.dram_tensor(f"xw_{g}", [128, KX, CPG], F32R, kind="ExternalInput")
            for g in "ico"}
    W_d = {m: nc.dram_tensor(f"W_{m}", [128, KH, CPG], F32R, kind="ExternalInput")
           for m in ("hi", "ci", "hc", "ho", "co")}
    ident_d = nc.dram_tensor("ident", [128, 128], F32R, kind="ExternalInput")
    onesrow_d = nc.dram_tensor("onesrow", [128, 128], F32R, kind="ExternalInput")
    hb1_d = nc.dram_tensor("hb1", [128, NA], F32R, kind="ExternalInput")
    ffw_d = nc.dram_tensor("ffw", [128, KF, FF], F32R, kind="ExternalInput")
    ffb_d = nc.dram_tensor("ffb", [128, MF], F32, kind="ExternalInput")
    hw_d = nc.dram_tensor("hw", [128, MF, NA], F32R, kind="ExternalInput")

    out_d = nc.dram_tensor("logits", [seq, NA], F32, kind="ExternalOutput")

    h_hist_f = nc.dram_tensor("h_hist_f", [CPG, seq], F32R)   # my h-dims x tokens
    h_hist_r = nc.dram_tensor("h_hist_r", [CPG, seq], F32R)   # token-reversed copy
    ag_out_f = nc.dram_tensor("ag_out_f", [2 * H, seq], F32R, addr_space="Shared")
    ag_out_r = nc.dram_tensor("ag_out_r", [2 * H, seq], F32R, addr_space="Shared")

    # ---------------- SBUF ----------------
    overlay_base = nc.sbuf_base  # FF tensors will overlay the recurrence weights
    xT_s = nc.alloc_sbuf_tensor("xT_s", [128, KX, seq], F32R)
    xw_s = {g: nc.alloc_sbuf_tensor(f"xw_{g}_s", [128, KX, CPG], F32R) for g in "ico"}
    W_s = {m: nc.alloc_sbuf_tensor(f"W_{m}_s", [128, KH, CPG], F32R)
           for m in ("hi", "ci", "hc", "ho", "co")}
    ident_s = nc.alloc_sbuf_tensor("ident_s", [128, 128], F32R)
    onesrow_s = nc.alloc_sbuf_tensor("onesrow_s", [128, 128], F32R)
    hb1_s = nc.alloc_sbuf_tensor("hb1_s", [128, NA], F32R)
    ffb_s = nc.alloc_sbuf_tensor("ffb_s", [128, MF], F32)

    xp_s = {g: nc.alloc_sbuf_tensor(f"xp_{g}_s", [128, len(tchunks), CPG], F32R)
            for g in "ico"}

    h_full = [nc.alloc_sbuf_tensor(f"h_full{p}", [128, KH], F32R) for p in range(2)]
    c_full = [nc.alloc_sbuf_tensor(f"c_full{p}", [128, KH], F32R) for p in range(2)]

    row = lambda name: nc.alloc_sbuf_tensor(name, [1, CPG], F32)
    it_r, wt_r, tmp_r, tmp2_r, tct_r, ot_r = (
        row("it_r"), row("wt_r"), row("tmp_r"), row("tmp2_r"), row("tct_r"), row("ot_r"))
    ct_r = [nc.alloc_sbuf_tensor(f"ct_r{p}", [1, CPG], F32R) for p in range(2)]
    ht_r = [nc.alloc_sbuf_tensor(f"ht_r{p}", [1, CPG], F32R) for p in range(2)]

    # FF stage tensors overlay the recurrence weight region (phases are
    # strictly sequential; loads gated on the recurrence finishing)
    off0 = (overlay_base + 31) & ~31
    ffw_s = nc.alloc_sbuf_tensor_at("ffw_s", [128, KF, FF], F32R, offset=off0)
    off0 += KF * FF * 4
    ag_s = nc.alloc_sbuf_tensor_at("ag_s", [128, KF, seq], F32R, offset=off0)
    off0 += KF * seq * 4
    hidT_s = nc.alloc_sbuf_tensor_at("hidT_s", [128, MF, seq], F32R, offset=off0)
    hw_s = nc.alloc_sbuf_tensor("hw_s", [128, MF, NA], F32R)
    log_s = nc.alloc_sbuf_tensor("log_s", [128, len(tchunks), NA], F32)

    pb = [nc.place_psum_tensor(f"pb{i}", [128, 448], F32, bank=i) for i in range(8)]

    sems = {}
    semnames = ["LD", "XPMM", "XPCP", "MM1", "MM2", "VCT", "VHT",
                "AC1", "ATCT", "AC2", "LCT", "LHT", "HIST", "PSEM", "BLS",
                "RCT", "RHT", "CC", "FFMM", "RELU", "HCPY", "LD2", "OUTD",
                "MMA", "MMB"]

    import contextlib
    ctx = contextlib.ExitStack()
    for s in semnames:
        sems[s] = ctx.enter_context(nc.semaphore(s))
    S = type("S", (), sems)

    load_list = [
        (xT_s.ap(), xT_d.ap()),
        (xw_s["i"].ap(), xw_d["i"].ap()), (xw_s["c"].ap(), xw_d["c"].ap()),
        (xw_s["o"].ap(), xw_d["o"].ap()),
        (W_s["hi"].ap(), W_d["hi"].ap()), (W_s["ci"].ap(), W_d["ci"].ap()),
        (W_s["hc"].ap(), W_d["hc"].ap()), (W_s["ho"].ap(), W_d["ho"].ap()),
        (W_s["co"].ap(), W_d["co"].ap()),
        (ident_s[:, :], ident_d[:, :]), (onesrow_s[:, :], onesrow_d[:, :]),
        (hb1_s[:, :], hb1_d[:, :]), (ffb_s[:, :], ffb_d[:, :]),
        (hw_s.ap(), hw_d.ap()),
    ]
    NLOADS = len(load_list)

    # xproj chains: list of (gate, tchunk_idx)
    xp_chains = [(g, i) for i in range(len(tchunks)) for g in "ico"]
    NXP = len(xp_chains)

    RINC = 48 if sim_solo else 6  # per-step increments of RCT/RHT

    block_ctx = nc.Block()
    block = block_ctx.__enter__()

    # =================== SYNC: loads + scatters + h_hist + output ===================
    @block.sync
    def _(sync):
        for dst, src in load_list:
            sync.dma_start(out=dst, in_=src).then_inc(S.LD, 16)

        with nc.allow_non_contiguous_dma(reason="h_hist column scatter"):
            for t in range(seq if stage >= 3 else 0):
                par = t % 2
                sync.wait_ge(S.VCT, t + 1)
                if t >= 2 and stage >= 4:
                    sync.wait_ge(S.BLS, 96 * (t - 1))
                sync.dma_start(
                    out=c_full[par][:, 0:3],
                    in_=bass.AP(ct_r[par], 0, [[CPG, 1], [3, 128], [1, 3]]),
                ).then_inc(S.LCT, 16)
                sync.wait_ge(S.VHT, t + 1)
                sync.dma_start(
                    out=h_full[par][:, 0:3],
                    in_=bass.AP(ht_r[par], 0, [[CPG, 1], [3, 128], [1, 3]]),
                ).then_inc(S.LHT, 16)
                if stage >= 4:
                    src = ht_r[par][0:1, :]
                    if skip_hist:
                        sync.dma_start(out=bass.AP(h_hist_f, 0, [[seq, 1], [1, CPG]]),
                                       in_=src).then_inc(S.HIST, 16)
                        sync.dma_start(out=bass.AP(h_hist_r, 0, [[seq, 1], [1, CPG]]),
                                       in_=src).then_inc(S.HIST, 16)
                    else:
                        sync.dma_start(out=bass.AP(h_hist_f, t, [[seq, CPG], [1, 1]]),
                                       in_=src).then_inc(S.HIST, 16)
                        sync.dma_start(
                            out=bass.AP(h_hist_r, seq - 1 - t, [[seq, CPG], [1, 1]]),
                            in_=src).then_inc(S.HIST, 16)

        # FF loads: ffw overlays the weight region -> wait until the
        # recurrence has fully consumed the weights
        if stage >= 2:
            sync.wait_ge(S.VHT, seq)
        sync.wait_ge(S.MM2, seq)
        sync.dma_start(out=ffw_s.ap(), in_=ffw_d.ap()).then_inc(S.LD2, 16)
        if stage >= 4:
            sync.wait_ge(S.CC, 32 if (sim_solo or local_cc) else 2)
        half = KF // 2
        sync.dma_start(
            out=ag_s[:, 0:half, :],
            in_=bass.AP(ag_out_f, 0, [[seq, 128], [128 * seq, half], [1, seq]]),
        ).then_inc(S.LD2, 16)
        sync.dma_start(
            out=ag_s[:, half:KF, :],
            in_=bass.AP(ag_out_r, H * seq, [[seq, 128], [128 * seq, half], [1, seq]]),
        ).then_inc(S.LD2, 16)
        # LD2 total: ffw (16) + ag halves (32) = 48
        # final output
        for i, (t0, tl) in enumerate(tchunks):
            sync.wait_ge(S.HCPY, i + 1)
            sync.dma_start(
                out=bass.AP(out_d, t0 * NA, [[NA, tl], [1, NA]]),
                in_=log_s[0:tl, i, :],
            ).then_inc(S.OUTD, 16)
        sync.wait_ge(S.OUTD, 16 * len(tchunks))

    # =================== TENSOR ===================
    @block.tensor
    def _(tensor):
        tensor.wait_ge(S.LD, 16 * NLOADS)

        # ---- x projections ----
        for idx, (g, i) in enumerate(xp_chains):
            b = idx % 8
            if idx >= 8:
                tensor.wait_ge(S.XPCP, idx - 7)
            t0, tl = tchunks[i]
            for k in range(KX):
                mm = tensor.matmul(pb[b][0:tl, 0:CPG],
                                   xT_s[:, k, t0:t0 + tl],
                                   xw_s[g][:, k, :],
                                   start=(k == 0), stop=(k == KX - 1),
                                   skip_group_check=True)
            mm.then_inc(S.XPMM, 1)

        tensor.wait_ge(S.XPCP, NXP)

        def wait_c(val16, val_other):
            # c_t-side dependency at the current stage
            if stage >= 4:
                tensor.wait_ge(S.RCT, RINC * val_other)
                tensor.wait_ge(S.LCT, 16 * val16)
            elif stage == 3:
                tensor.wait_ge(S.LCT, 16 * val16)
            elif stage == 2:
                tensor.wait_ge(S.VCT, val_other)
            elif stage == 1:
                tensor.wait_ge(S.AC1, val_other)

        def wait_h(val16, val_other):
            if stage >= 4:
                tensor.wait_ge(S.RHT, RINC * val_other)
                tensor.wait_ge(S.LHT, 16 * val16)
            elif stage == 3:
                tensor.wait_ge(S.LHT, 16 * val16)
            elif stage == 2:
                tensor.wait_ge(S.VHT, val_other)
            elif stage == 1:
                tensor.wait_ge(S.AC2, val_other)

        # ---- recurrence ----
        for t in range(seq):
            par = t % 2
            prev = (t - 1) % 2
            B = [pb[4 * par + i] for i in range(4)]  # b0..b3 this parity
            if t >= 2 and stage >= 2:
                tensor.wait_ge(S.VHT, t - 1)

            tci = t // 128
            trow = t % 128
            tlc = tchunks[tci][1]

            # b0: c2i (runs in the ht-exchange shadow) + h2i + xp_i select
            if t >= 1:
                wait_c(t, t)
                for k in range(KH):
                    tensor.matmul(B[0][0:1, 0:CPG], c_full[prev][:, k:k + 1],
                                  W_s["ci"][:, k, :],
                                  start=(k == 0), stop=False,
                                  tile_position=(0, 0), skip_group_check=True)
                wait_h(t, t)
                for k in range(KH):
                    tensor.matmul(B[0][0:1, 0:CPG], h_full[prev][:, k:k + 1],
                                  W_s["hi"][:, k, :],
                                  start=False, stop=False,
                                  tile_position=(0, 0), skip_group_check=True)
            tensor.matmul(B[0][0:1, 0:CPG], ident_s[0:tlc, trow:trow + 1],
                          xp_s["i"][0:tlc, tci, :],
                          start=(t == 0), stop=True,
                          tile_position=(0, 0), skip_group_check=True
                          ).then_inc(S.MMA, 1)

            # b2: h2c + xp_c select
            if t >= 1:
                for k in range(KH):
                    tensor.matmul(B[2][0:1, 0:CPG], h_full[prev][:, k:k + 1],
                                  W_s["hc"][:, k, :],
                                  start=(k == 0), stop=False,
                                  tile_position=(0, 0), skip_group_check=True)
            tensor.matmul(B[2][0:1, 0:CPG], ident_s[0:tlc, trow:trow + 1],
                          xp_s["c"][0:tlc, tci, :],
                          start=(t == 0), stop=True,
                          tile_position=(0, 0), skip_group_check=True
                          ).then_inc(S.MMB, 1)

            # b3 part 1: h2o + xp_o select (accumulation group stays open);
            # overlaps the ct exchange
            if t >= 1:
                for k in range(KH):
                    tensor.matmul(B[3][0:1, 0:CPG], h_full[prev][:, k:k + 1],
                                  W_s["ho"][:, k, :],
                                  start=(k == 0), stop=False,
                                  tile_position=(0, 0), skip_group_check=True)
            tensor.matmul(B[3][0:1, 0:CPG], ident_s[0:tlc, trow:trow + 1],
                          xp_s["o"][0:tlc, tci, :],
                          start=(t == 0), stop=False,
                          tile_position=(0, 0), skip_group_check=True)

            # b3 part 2: c2o on fresh ct
            wait_c(t + 1, t + 1)
            for k in range(KH):
                mm = tensor.matmul(B[3][0:1, 0:CPG], c_full[par][:, k:k + 1],
                                   W_s["co"][:, k, :],
                                   start=False, stop=(k == KH - 1),
                                   tile_position=(0, 0), skip_group_check=True)
            mm.then_inc(S.MM2, 1)

        # ---- FF ----
        if stage >= 2:
            tensor.wait_ge(S.VHT, seq)
        tensor.wait_ge(S.LD2, 48)
        for m in range(MF):
            for k in range(KF):
                mm = tensor.matmul(pb[m][0:128, 0:seq],
                                   ffw_s[:, k, 128 * m:128 * (m + 1)],
                                   ag_s[:, k, :],
                                   start=(k == 0), stop=(k == KF - 1),
                                   skip_group_check=True)
            mm.then_inc(S.FFMM, 1)
        tensor.wait_ge(S.RELU, MF)
        for i, (t0, tl) in enumerate(tchunks):
            b = pb[6 + (i % 2)]
            if i >= 2:
                tensor.wait_ge(S.HCPY, i - 1)
            for k in range(MF):
                tensor.matmul(b[0:tl, 0:NA], hidT_s[:, k, t0:t0 + tl],
                              hw_s[:, k, :],
                              start=(k == 0), stop=False, skip_group_check=True)
            tensor.matmul(b[0:tl, 0:NA], onesrow_s[:, 0:tl], hb1_s[:, :],
                          start=False, stop=True, skip_group_check=True
                          ).then_inc(S.MM1, 1)  # head-chain-done

    # =================== VECTOR ===================
    @block.vector
    def _(vector):
        # xproj copies
        for idx, (g, i) in enumerate(xp_chains):
            b = idx % 8
            vector.wait_ge(S.XPMM, idx + 1)
            t0, tl = tchunks[i]
            vector.tensor_copy(xp_s[g][0:tl, i, :],
                               pb[b][0:tl, 0:CPG]).then_inc(S.XPCP, 1)

        for t in range(seq if stage >= 2 else 0):
            par = t % 2
            prev = (t - 1) % 2
            vector.wait_ge(S.AC1, t + 1)
            if t >= 2 and stage >= 3:
                vector.wait_ge(S.LCT, 16 * t)
            if t == 0:
                vector.tensor_mul(ct_r[0][0:1, :], it_r[0:1, :], wt_r[0:1, :]
                                  ).then_inc(S.VCT, 1)
            else:
                vector.tensor_sub(tmp_r[0:1, :], wt_r[0:1, :],
                                  ct_r[prev][0:1, :].bitcast(mybir.dt.float32))
                vector.drain()
                vector.tensor_mul(tmp2_r[0:1, :], it_r[0:1, :], tmp_r[0:1, :])
                vector.drain()
                vector.tensor_add(ct_r[par][0:1, :], tmp2_r[0:1, :],
                                  ct_r[prev][0:1, :].bitcast(mybir.dt.float32)
                                  ).then_inc(S.VCT, 1)
            vector.wait_ge(S.ATCT, t + 1)
            vector.wait_ge(S.AC2, t + 1)
            if t >= 2 and stage >= 3:
                vector.wait_ge(S.LHT, 16 * t)
                if stage >= 4:
                    vector.wait_ge(S.HIST, 32 * t)
            vector.tensor_mul(ht_r[par][0:1, :], ot_r[0:1, :], tct_r[0:1, :]
                              ).then_inc(S.VHT, 1)

        # head copies
        for i, (t0, tl) in enumerate(tchunks):
            b = pb[6 + (i % 2)]
            vector.wait_ge(S.MM1, i + 1)
            vector.tensor_copy(log_s[0:tl, i, :], b[0:tl, 0:NA]).then_inc(S.HCPY, 1)

    # =================== SCALAR (ACT) ===================
    @block.scalar
    def _(scalar):
        for t in range(seq if stage >= 1 else 0):
            par = t % 2
            B = [pb[4 * par + i] for i in range(4)]
            scalar.wait_ge(S.MMA, t + 1)
            if t >= 1 and stage >= 2:
                scalar.wait_ge(S.VCT, t)
            scalar.activation(it_r[0:1, :], B[0][0:1, 0:CPG], AF.Sigmoid)
            scalar.wait_ge(S.MMB, t + 1)
            scalar.activation(wt_r[0:1, :], B[2][0:1, 0:CPG], AF.Tanh
                              ).then_inc(S.AC1, 1)
            if stage >= 2:
                scalar.wait_ge(S.VCT, t + 1)
            scalar.activation(tct_r[0:1, :], ct_r[par][0:1, :].bitcast(mybir.dt.float32),
                              AF.Tanh).then_inc(S.ATCT, 1)
            scalar.wait_ge(S.MM2, t + 1)
            scalar.activation(ot_r[0:1, :], B[3][0:1, 0:CPG], AF.Sigmoid
                              ).then_inc(S.AC2, 1)

        # FF relu with per-partition bias
        for m in range(MF):
            scalar.wait_ge(S.FFMM, m + 1)
            scalar.activation(hidT_s[:, m, :],
                              pb[m][0:128, 0:seq], AF.Relu,
                              bias=ffb_s[:, m:m + 1]).then_inc(S.RELU, 1)

    # =================== GPSIMD: remote exchange (prep early, trigger late) ===================
    @block.gpsimd
    def _(gpsimd):
        gpsimd.wait_ge(S.LD, 16 * NLOADS)
        for t in range(seq if stage >= 4 else 0):
            par = t % 2
            if sim_solo:
                # Timing stand-in: one local DMA per exchange models
                # trigger+transfer+sem-prop; sem totals match the real build.
                gpsimd.wait_ge(S.LCT, 16 * (t + 1))
                gpsimd.dma_start(
                    out=c_full[par][:, 3:6], in_=c_full[par][:, 0:3],
                ).then_inc(S.RCT, 48).then_inc(S.BLS, 48)
                gpsimd.wait_ge(S.LHT, 16 * (t + 1))
                gpsimd.dma_start(
                    out=h_full[par][:, 3:6], in_=h_full[par][:, 0:3],
                ).then_inc(S.RHT, 48).then_inc(S.BLS, 48)
                continue
            # Descriptor generation for this step's 6 broadcasts happens here,
            # off the critical path; only the triggers wait for data.
            for buf, rsem in ((c_full, S.RCT), (h_full, S.RHT)):
                for delta in (1, 2, 3):
                    rd = [None] * 8
                    rd[delta] = (0, 0 if self_loop else delta)
                    gpsimd.remote_dma_broadcast(
                        out_ap=buf[par][:, 3 * delta:3 * delta + 3],
                        in_ap=buf[par][:, 0:3],
                        remote_sem=rsem, local_sem=S.BLS,
                        rdests=rd,
                    ).then_inc(S.PSEM, 1)
            gpsimd.wait_ge(S.PSEM, 6 * t + 3)
            gpsimd.wait_ge(S.LCT, 16 * (t + 1))
            gpsimd.trigger_dma(3)
            gpsimd.wait_ge(S.PSEM, 6 * t + 6)
            gpsimd.wait_ge(S.LHT, 16 * (t + 1))
            gpsimd.trigger_dma(3)

        if stage >= 4:
            gpsimd.wait_ge(S.HIST, 32 * seq)
            if sim_solo or local_cc:
                gpsimd.dma_start(
                    out=bass.AP(ag_out_f, 0, [[seq, CPG], [1, seq]]),
                    in_=h_hist_f.ap(),
                ).then_inc(S.CC, 16)
                gpsimd.dma_start(
                    out=bass.AP(ag_out_r, 0, [[seq, CPG], [1, seq]]),
                    in_=h_hist_r.ap(),
                ).then_inc(S.CC, 16)
            else:
                gpsimd.collective_compute(
                    "AllGather",
                    mybir.AluOpType.bypass,
                    replica_groups=[list(range(8))],
                    ins=[h_hist_f.ap().opt()],
                    outs=[ag_out_f.ap().opt()],
                ).then_inc(S.CC)
                gpsimd.collective_compute(
                    "AllGather",
                    mybir.AluOpType.bypass,
                    replica_groups=[list(range(8))],
                    ins=[h_hist_r.ap().opt()],
                    outs=[ag_out_r.ap().opt()],
                ).then_inc(S.CC)

    block_ctx.__exit__(None, None, None)
    ctx.close()
    nc.compile()
    return nc


# ---------------- host-side helpers ----------------

def _perm_index(g):
    # E[p, k] = h-space index stored at SBUF [partition p, chunk k] on core
    # with group index g: slot s = k//3 holds the slab of group g ^ s.
    p = np.arange(128)[:, None]
    k = np.arange(KH)[None, :]
    return 384 * ((g ^ (k // 3)) & 3) + 3 * p + (k % 3)


def _prep_core_inputs(c, inp):
    pre = "lr" if c < G else "rl"
    g = c % G
    Sl = slice(CPG * g, CPG * (g + 1))
    E = _perm_index(g)

    x = np.asarray(inp["x"], np.float32)
    xin = x if pre == "lr" else x[::-1]
    seq = x.shape[0]

    x_aug = np.zeros((KX * 128, seq), np.float32)
    x_aug[:D_IN] = xin.T
    x_aug[D_IN] = 1.0
    xT = np.ascontiguousarray(x_aug.reshape(KX, 128, seq).transpose(1, 0, 2))

    def aug_w(w, b):
        wa = np.zeros((KX * 128, CPG), np.float32)
        wa[:D_IN] = w[:, Sl]
        wa[D_IN] = b[Sl]
        return np.ascontiguousarray(wa.reshape(KX, 128, CPG).transpose(1, 0, 2))

    def perm_w(w):
        return np.ascontiguousarray(np.asarray(w, np.float32)[:, Sl][E])

    d = {
        "xT": xT,
        "xw_i": aug_w(inp[pre + "_x2i"], inp[pre + "_bi"]),
        "xw_c": aug_w(inp[pre + "_x2c"], inp[pre + "_bc"]),
        "xw_o": aug_w(inp[pre + "_x2o"], inp[pre + "_bo"]),
        "W_hi": perm_w(inp[pre + "_h2i"]),
        "W_ci": perm_w(inp[pre + "_c2i"]),
        "W_hc": perm_w(inp[pre + "_h2c"]),
        "W_ho": perm_w(inp[pre + "_h2o"]),
        "W_co": perm_w(inp[pre + "_c2o"]),
        "ident": np.eye(128, dtype=np.float32),
        "onesrow": np.zeros((128, 128), np.float32),
        "hb1": np.zeros((128, NA), np.float32),
        "ffw": np.ascontiguousarray(
            np.asarray(inp["ff_w"], np.float32).reshape(KF, 128, FF).transpose(1, 0, 2)),
        "ffb": np.ascontiguousarray(
            np.asarray(inp["ff_b"], np.float32).reshape(MF, 128).T),
        "hw": np.ascontiguousarray(
            np.asarray(inp["head_w"], np.float32).reshape(MF, 128, NA).transpose(1, 0, 2)),
    }
    d["onesrow"][0, :] = 1.0
    d["hb1"][0, :] = np.asarray(inp["head_b"], np.float32)
    return d


def kernel(**inputs):
    from concourse.bass_utils import run_bass_kernel_spmd

    seq = np.asarray(inputs["x"]).shape[0]
    if "nc" not in _CACHE or _CACHE.get("seq") != seq:
        _CACHE["nc"] = _build(seq)
        _CACHE["seq"] = seq
    nc = _CACHE["nc"]

    in_maps = [_prep_core_inputs(c, inputs) for c in range(8)]
    res = run_bass_kernel_spmd(nc, in_maps, core_ids=list(range(8)), trace=False)
    return res.results[0]["logits"]
